# revision 14
# baseline (speedup 1.0000x reference)
"""Trainium2 Bass kernel for nn_CausalSelfAttention (BitLinear QKV/O + RoPE + causal attn).

Sharding: 2 heads x 2 batches per core (head-parallel), bf16 throughout.
Per core: q/k/v projections ([d, t] layout, bf16 matmuls, fp32 PSUM), RoPE via
DMA partition shuffle + DVE/Pool elementwise, scores in [k, q] layout, exp on
ACT (scale=0.125) -> E bf16, pv as many small-N matmuls with E as the
stationary operand giving [q, d_aug] output whose 65th column is the softmax
denominator (ones column baked into v_aug), per-partition normalization via
DVE tensor_tensor with a broadcast reciprocal, PE transposes back to [d, t],
column-sharded output projection producing a bf16 partial [4096, 1024] that
the host sums across cores.

Engine budget per core (TimelineSim model): PE ~102us (at the matmul floor:
cost = out-free-size x 0.417ns/row), ACT ~85us (exp + some copies), DVE ~85us
(copies, rope, epilogue), Pool ~45us (tri masks, rope adds). Emission order
interleaves batch-0 attention with batch-1 projections (and batch-1 attention
with batch-0 out-projection) so exp overlaps projection matmuls.
"""
import sys

sys.path.insert(0, "/opt/trn_rl_repo")

import ml_dtypes
import numpy as np

BF16 = ml_dtypes.bfloat16
GROUP = 128
N_HEADS = 16
EPS = 1e-8
B, T, C = 2, 2048, 1024
HD = 64
N_CORES = 8
HPC = N_HEADS // N_CORES  # 2 heads per core


# ---------------------------------------------------------------- host prep
def _ternary_quantize(w):
    O, I = w.shape
    g = w.reshape(O, I // GROUP, GROUP).astype(np.float32)
    scale = np.maximum(np.mean(np.abs(g), axis=-1, keepdims=True), EPS).astype(
        np.float32
    )
    wn = g / scale
    q = np.where(wn > 0.5, 1.0, np.where(wn < -0.5, -1.0, 0.0)).astype(np.float32)
    return (q * scale).reshape(O, I).astype(np.float32)


def _make_core_inputs(x, wq, wk, wv, wo, rope_cos, rope_sin):
    """Returns list of 8 per-core input dicts (bf16 device layouts)."""
    x = np.ascontiguousarray(x.astype(np.float32).reshape(B * T, C))
    wq_q = _ternary_quantize(wq)
    wk_q = _ternary_quantize(wk)
    wv_q = _ternary_quantize(wv)
    wo_q = _ternary_quantize(wo)

    xT = x.T  # [1024 c, 4096 t]
    xt_slab = np.ascontiguousarray(
        xT.reshape(8, 128, 8, 512).transpose(2, 1, 0, 3)
    ).astype(BF16)  # [s, p, kk, u]

    cosT = rope_cos.astype(np.float32).T  # [32, 2048]
    sinT = rope_sin.astype(np.float32).T
    cos_t = np.tile(cosT, (4, 1)).astype(BF16)
    sin_t = np.concatenate([-sinT, sinT, -sinT, sinT], axis=0).astype(BF16)
    # strict upper triangle (invalid: key k > query q within a diagonal block)
    tri = (np.arange(128)[None, :] < np.arange(128)[:, None]).astype(BF16)
    ident = np.eye(128, dtype=np.float32).astype(BF16)
    negid = (-1000.0 * np.eye(128, dtype=np.float32)).astype(BF16)

    maps = []
    for core in range(N_CORES):
        r0 = core * HPC * HD
        rows = slice(r0, r0 + HPC * HD)

        def w_lhsT(w_qq):
            wsT = w_qq[rows, :].T  # [1024 in, 128 d]
            return np.ascontiguousarray(
                wsT.reshape(8, 128, 128).transpose(1, 0, 2)
            ).astype(BF16)  # [p, kk, d]

        woc = wo_q[:, rows]  # [1024 o, 128 d]
        maps.append(
            {
                "xt": xt_slab,
                "wqT": w_lhsT(wq_q),
                "wkT": w_lhsT(wk_q),
                "wvT": w_lhsT(wv_q),
                "woC": np.ascontiguousarray(woc.T).astype(BF16),  # [128 d, 1024 o]
                "cos_t": cos_t,
                "sin_t": sin_t,
                "tri": tri,
                "ident": ident,
                "negid": negid,
            }
        )
    return maps


# ---------------------------------------------------------------- BIR post-pass
def _split_excess_waits(nc, max_waits=1):
    """walrus CoreV3 codegen rejects instructions with >1 sem wait; split the
    excess into preceding NoOps on the same engine."""
    import concourse.mybir as mybir

    for f in nc.m.functions:
        for bb in f.blocks:
            insts = bb.instructions
            i = 0
            while i < len(insts):
                ins = insts[i]
                si = ins.sync_info
                if si is not None and si.on_wait and len(si.on_wait) > max_waits:
                    waits = list(si.on_wait)
                    si.on_wait = waits[:max_waits]
                    rest = waits[max_waits:]
                    new_ops = []
                    for j in range(0, len(rest), max_waits):
                        new_ops.append(
                            mybir.InstNoOp(
                                name=nc.get_next_instruction_name(),
                                sync_info=mybir.SyncInfo(
                                    on_wait=rest[j : j + max_waits], on_update=[]
                                ),
                                bass_nofuse=True,
                                engine=ins.engine,
                            )
                        )
                    insts[i:i] = new_ops
                    i += len(new_ops)
                i += 1


# ---------------------------------------------------------------- device kernel
def _emit(nc, tc, d):
    import concourse.mybir as mybir
    from concourse.bass import ds, ts

    f32 = mybir.dt.float32
    bf16 = mybir.dt.bfloat16
    AF = mybir.ActivationFunctionType
    OP = mybir.AluOpType

    with nc.allow_low_precision(
        reason="bf16 activations; fp32 accum in PSUM; 2e-2 rel tol"
    ), tc.tile_pool(name="const", bufs=1) as cp, tc.tile_pool(
        name="persist", bufs=1
    ) as pp, tc.tile_pool(name="xt", bufs=3) as xtp, tc.tile_pool(
        name="sw", bufs=2
    ) as swp, tc.tile_pool(name="tmp", bufs=2) as tmpp, tc.tile_pool(
        name="E", bufs=3
    ) as epool, tc.tile_pool(name="y2n", bufs=2) as y2np, tc.tile_pool(
        name="rc", bufs=4
    ) as rcp, tc.tile_pool(name="scP", bufs=2, space="PSUM") as scp, tc.tile_pool(
        name="ypP", bufs=2, space="PSUM"
    ) as ypp, tc.tile_pool(name="scrP", bufs=2, space="PSUM") as scr:
        # ---- constants (xt slab 0 is DMA'd first, in the schedule below;
        # weight order matches first use so the PE start isn't DMA-gated)
        wq_t = cp.tile([128, 8, 128], bf16)
        wk_t = cp.tile([128, 8, 128], bf16)
        wv_t = cp.tile([128, 8, 128], bf16)
        woC = cp.tile([128, 1024], bf16)
        cos_sb = cp.tile([128, 2048], bf16)
        sin_sb = cp.tile([128, 2048], bf16)
        tri_t = cp.tile([128, 128], bf16)
        id_t = cp.tile([128, 128], bf16)
        nid_t = cp.tile([128, 128], bf16)

        def emit_consts_early():
            nc.sync.dma_start(wq_t[:], d["wqT"])
            nc.sync.dma_start(wk_t[:], d["wkT"])
            nc.sync.dma_start(wv_t[:], d["wvT"])

        def emit_consts_late():
            nc.sync.dma_start(cos_sb[:], d["cos_t"])
            nc.sync.dma_start(sin_sb[:], d["sin_t"])
            nc.sync.dma_start(tri_t[:], d["tri"])
            nc.sync.dma_start(id_t[:], d["ident"])
            nc.sync.dma_start(nid_t[:], d["negid"])
            nc.sync.dma_start(woC[:], d["woC"])

        # ---- persistent tensors
        qT = pp.tile([128, 4096], bf16)
        kT = pp.tile([128, 4096], bf16)
        vT = pp.tile([128, 4096], bf16)
        v_aug = pp.tile([128, 32 * 129], bf16)  # per key-block: [v0(64)|1|v1(64)]
        y2T = pp.tile([128, 4096], bf16)
        bo = [pp.tile([128, 16384], bf16, name=f"bo{b}") for b in range(2)]
        # bake the shared ones columns (col 64 of every 129-block)
        nc.gpsimd.memset(v_aug[:], 1.0)

        # ---- phase A granules: xt prefetch, then per-projection granules.
        # q/k granules include RoPE on the slab's columns; the v granule
        # includes the v transposes — so attention on a slab's queries can
        # start right after its three projection granules.
        _xt_tiles = {}

        def g_xt(s):
            def f():
                t = xtp.tile([128, 8, 512], bf16, tag="xt")
                nc.sync.dma_start(t[:], d["xt"][s])
                _xt_tiles[s] = t
            return f

        def _rope(tns, s):
            u = (s % 4) * 512  # within-batch token offset
            ccols = ds(u, 512)
            scols = ds(s * 512, 512)
            sw = swp.tile([128, 512], bf16, tag="sw")
            nc.sync.dma_start(sw[0:32, :], tns[32:64, scols])
            nc.sync.dma_start(sw[32:64, :], tns[0:32, scols])
            nc.sync.dma_start(sw[64:96, :], tns[96:128, scols])
            nc.sync.dma_start(sw[96:128, :], tns[64:96, scols])
            tmp = tmpp.tile([128, 512], bf16, tag="tmp")
            nc.vector.tensor_tensor(tmp[:], tns[:, scols], cos_sb[:, ccols], OP.mult)
            nc.vector.tensor_tensor(sw[:], sw[:], sin_sb[:, ccols], OP.mult)
            eng = nc.vector if s == 0 else nc.gpsimd
            eng.tensor_tensor(tns[:, scols], tmp[:], sw[:], OP.add)

        def g_proj(s, which, qk_on_act=False):
            def f():
                xt_t = _xt_tiles[s]
                w_t, dest = ((wq_t, qT), (wk_t, kT), (wv_t, vT))[which]
                ps = scr.tile([128, 512], f32, tag="scr")
                for kk in range(8):
                    nc.tensor.matmul(
                        ps[:],
                        w_t[:, kk, :],
                        xt_t[:, kk, :],
                        start=(kk == 0),
                        stop=(kk == 7),
                    )
                if qk_on_act and which < 2:
                    nc.scalar.copy(dest[:, ts(s, 512)], ps[:])
                else:
                    nc.vector.tensor_copy(dest[:, ts(s, 512)], ps[:])
                if which < 2:
                    _rope(dest, s)
                else:
                    for blk in range(4):
                        g = s * 4 + blk
                        tp = scr.tile([128, 128], bf16, tag="scr")
                        nc.tensor.transpose(
                            tp[:], vT[:, ds(g * 128, 128)], id_t[:]
                        )
                        nc.vector.tensor_copy(
                            v_aug[:, ds(g * 129, 64)], tp[:, 0:64]
                        )
                        nc.vector.tensor_copy(
                            v_aug[:, ds(g * 129 + 65, 64)], tp[:, 64:128]
                        )
            return f

        # ---- filler queue: independent PE work popped between attention
        # pipeline units so the in-order PE stream never starves while ACT
        # works through the exp backlog. Items are (slab_done_marker, fn).
        filler = []

        def pop_filler(n=1):
            for _ in range(n):
                if filler:
                    filler.pop(0)[1]()

        def drain_slab(s):
            while any(m is not None and m <= s for m, _ in filler):
                filler.pop(0)[1]()

        # ---- phase B: one (batch, 512-query-chunk) of attention
        def emit_chunk(b, qi):
            nj = 4 * qi + 4
            npairs = nj // 2
            yp = [ypp.tile([128, 260], f32, tag="yp", name=f"yp{b}_{qi}_{h}")
                  for h in range(2)]
            q0 = b * 2048 + qi * 512

            units = [(p, h) for p in range(npairs) for h in range(2)]

            def emit_sc(p, h):
                sc = scp.tile([128, 1024], f32, tag="sc", name=f"sc{b}_{qi}_{p}_{h}")
                for jj in range(2):
                    j = 2 * p + jj
                    dlt0 = max(j * 128 - qi * 512, 0)
                    nc.tensor.matmul(
                        sc[:, ds(jj * 512 + dlt0, 512 - dlt0)],
                        kT[64 * h : 64 * h + 64, ds(b * 2048 + j * 128, 128)],
                        qT[64 * h : 64 * h + 64, ds(q0 + dlt0, 512 - dlt0)],
                        start=True,
                        stop=True,
                    )
                    if j >= 4 * qi:
                        # causal mask: add -1000 to the strict upper triangle
                        # of the diagonal block so exp() flushes it to zero
                        qbl = j - 4 * qi
                        nc.tensor.matmul(
                            sc[:, ds(jj * 512 + qbl * 128, 128)],
                            nid_t[:],
                            tri_t[:],
                            start=False,
                            stop=True,
                            skip_group_check=True,
                        )
                return sc

            def emit_rest(p, h, sc):
                # exp (trim the diagonal pairs so unwritten PSUM is never read)
                E = epool.tile([128, 1024], bf16, tag="E")
                j0, j1 = 2 * p, 2 * p + 1
                d0 = max(j0 * 128 - qi * 512, 0)
                d1 = max(j1 * 128 - qi * 512, 0)
                if d0 == 0 and d1 == 0:
                    nc.scalar.activation(E[:], sc[:], AF.Exp, scale=0.125)
                else:
                    nc.scalar.activation(
                        E[:, ds(d0, 512 - d0)], sc[:, ds(d0, 512 - d0)],
                        AF.Exp, scale=0.125,
                    )
                    nc.scalar.activation(
                        E[:, ds(512 + d1, 512 - d1)], sc[:, ds(512 + d1, 512 - d1)],
                        AF.Exp, scale=0.125,
                    )
                # pv: E block is the stationary operand -> out [q, d_aug]
                for jj, j in ((0, j0), (1, j1)):
                    g = b * 16 + j
                    for qbl in range(4):
                        qb_g = qi * 4 + qbl
                        if qb_g < j:
                            continue
                        nc.tensor.matmul(
                            yp[h][:, ds(qbl * 65, 65)],
                            E[:, ds(jj * 512 + qbl * 128, 128)],
                            v_aug[:, ds(g * 129 + 64 * h, 65)],
                            start=(j == 0),
                            stop=(j == qb_g),
                            skip_group_check=True,
                        )

            prev = None
            for ui, u in enumerate(units):
                sc = emit_sc(*u)
                if prev is not None:
                    emit_rest(prev[0], prev[1], prev[2])
                prev = (u[0], u[1], sc)
                if ui % 2 == 1:
                    pop_filler(1)
            emit_rest(prev[0], prev[1], prev[2])

            # epilogue: normalize by the denominator column, then transpose
            y2n = y2np.tile([128, 512], bf16, tag="y2n")
            for h in range(2):
                ypr = yp[h][:].rearrange("p (a c) -> p a c", a=4)
                rc = rcp.tile([128, 4], f32, tag="rc")
                den = ypr[:, :, 64:65] if h == 0 else ypr[:, :, 0:1]
                nc.vector.reciprocal(rc[:], den)
                data = ypr[:, :, 0:64] if h == 0 else ypr[:, :, 1:65]
                rcb = rc[:].unsqueeze(2).broadcast_to([128, 4, 64])
                outap = y2n[:].rearrange("p (a c) -> p a c", a=4)[
                    :, :, 64 * h : 64 * h + 64
                ]
                nc.vector.tensor_tensor(outap, data, rcb, OP.mult)
            for qbl in range(4):
                tp = scr.tile([128, 128], bf16, tag="scr")
                nc.tensor.transpose(tp[:], y2n[:, ds(qbl * 128, 128)], id_t[:])
                nc.vector.tensor_copy(
                    y2T[:, ds(b * 2048 + (qi * 4 + qbl) * 128, 128)], tp[:]
                )

        # ---- phase C: output projection granules (one token-block each)
        _copy_ctr = [0]

        def g_outblk(b, tb, act_mod=0):
            def f():
                for oc in range(2):
                    op = scr.tile([128, 512], f32, tag="scr")
                    nc.tensor.matmul(
                        op[:],
                        y2T[:, ds(b * 2048 + tb * 128, 128)],
                        woC[:, ds(oc * 512, 512)],
                        start=True,
                        stop=True,
                    )
                    o0 = tb * 1024 + oc * 512
                    nc.vector.tensor_copy(bo[b][:, ds(o0, 256)], op[:, 0:256])
                    nc.scalar.copy(bo[b][:, ds(o0 + 256, 256)], op[:, 256:512])
            return f

        def g_outdma(b, grp, ntb=4):
            def f():
                dram = d["outp"][
                    ds(b * 2048 + grp * ntb * 128, ntb * 128), :
                ].rearrange("(a p) c -> p a c", p=128)
                src = bo[b][:, ds(grp * ntb * 1024, ntb * 1024)].rearrange(
                    "p (a c) -> p a c", a=ntb
                )
                nc.sync.dma_start(dram, src)
            return f

        # ---------------- emission schedule ----------------
        # slab s feeds chunk (s//4, s%4); attention starts right after slab 0.
        g_xt(0)()
        emit_consts_early()
        g_xt(1)()
        emit_consts_late()
        for w in range(3):
            g_proj(0, w, qk_on_act=True)()
        # filler: remaining slabs (xt prefetched one slab ahead) ...
        for s in range(1, 8):
            if s + 1 < 8:
                filler.append((None, g_xt(s + 1)))
            for w in range(3):
                filler.append((s, g_proj(s, w, qk_on_act=(s == 1))))
        # ... then batch-0 out-projection blocks
        for grp in range(4):
            for tb in range(grp * 4, grp * 4 + 4):
                filler.append((None, g_outblk(0, tb, act_mod=4)))
            filler.append((None, g_outdma(0, grp)))

        for qi in range(4):
            drain_slab(qi)
            emit_chunk(0, qi)
        for qi in range(4):
            drain_slab(4 + qi)
            emit_chunk(1, qi)
            if qi < 3:
                for tb in range(qi * 4, qi * 4 + 4):
                    filler.append((None, g_outblk(1, tb)))
                    if tb % 2 == 1:
                        filler.append((None, g_outdma(1, tb // 2, ntb=2)))
        pop_filler(len(filler))
        for tb in range(12, 16):
            g_outblk(1, tb)()
            if tb % 2 == 1:
                g_outdma(1, tb // 2, ntb=2)()


_NC_CACHE = {}


def _build():
    if "nc" in _NC_CACHE:
        return _NC_CACHE["nc"]
    import concourse.bass as bass
    import concourse.mybir as mybir
    import concourse.tile as tile

    bf16 = mybir.dt.bfloat16
    nc = bass.Bass("TRN2", target_bir_lowering=False, debug=False, num_devices=1)
    d = {
        "xt": nc.dram_tensor("xt", [8, 128, 8, 512], bf16, kind="ExternalInput").ap(),
        "wqT": nc.dram_tensor("wqT", [128, 8, 128], bf16, kind="ExternalInput").ap(),
        "wkT": nc.dram_tensor("wkT", [128, 8, 128], bf16, kind="ExternalInput").ap(),
        "wvT": nc.dram_tensor("wvT", [128, 8, 128], bf16, kind="ExternalInput").ap(),
        "woC": nc.dram_tensor("woC", [128, 1024], bf16, kind="ExternalInput").ap(),
        "cos_t": nc.dram_tensor("cos_t", [128, 2048], bf16, kind="ExternalInput").ap(),
        "sin_t": nc.dram_tensor("sin_t", [128, 2048], bf16, kind="ExternalInput").ap(),
        "tri": nc.dram_tensor("tri", [128, 128], bf16, kind="ExternalInput").ap(),
        "ident": nc.dram_tensor("ident", [128, 128], bf16, kind="ExternalInput").ap(),
        "negid": nc.dram_tensor("negid", [128, 128], bf16, kind="ExternalInput").ap(),
        "outp": nc.dram_tensor("outp", [4096, 1024], bf16, kind="ExternalOutput").ap(),
    }
    with tile.TileContext(nc) as tc:
        _emit(nc, tc, d)
    _split_excess_waits(nc)
    _NC_CACHE["nc"] = nc
    return nc


def kernel(x, wq, wk, wv, wo, rope_cos, rope_sin):
    from concourse import bass_utils

    x, wq, wk, wv, wo, rope_cos, rope_sin = (
        np.asarray(a, dtype=np.float32)
        for a in (x, wq, wk, wv, wo, rope_cos, rope_sin)
    )
    in_maps = _make_core_inputs(x, wq, wk, wv, wo, rope_cos, rope_sin)
    nc = _build()
    res = bass_utils.run_bass_kernel_spmd(nc, in_maps, core_ids=list(range(N_CORES)))
    total = np.zeros((B * T, C), np.float32)
    for i in range(N_CORES):
        total += res.results[i]["outp"].astype(np.float32)
    return total.reshape(B, T, C).astype(np.float32)


# revision 15
# speedup vs baseline: 1.1280x; 1.1280x over previous
"""Trainium2 Bass kernel for nn_CausalSelfAttention (BitLinear QKV/O + RoPE + causal attn).

Sharding: 2 heads x 2 batches per core (head-parallel), bf16 throughout.
Per core: q/k/v projections ([d, t] layout, bf16 matmuls, fp32 PSUM), RoPE via
DMA partition shuffle + DVE/Pool elementwise, scores in [k, q] layout, exp on
ACT (scale=0.125) -> E bf16, pv as many small-N matmuls with E as the
stationary operand giving [q, d_aug] output whose 65th column is the softmax
denominator (ones column baked into v_aug), per-partition normalization via
DVE tensor_tensor with a broadcast reciprocal, PE transposes back to [d, t],
column-sharded output projection producing a bf16 partial [4096, 1024] that
the host sums across cores.

Engine budget per core (TimelineSim model): PE ~102us (at the matmul floor:
cost = out-free-size x 0.417ns/row), ACT ~85us (exp + some copies), DVE ~85us
(copies, rope, epilogue), Pool ~45us (tri masks, rope adds). Emission order
interleaves batch-0 attention with batch-1 projections (and batch-1 attention
with batch-0 out-projection) so exp overlaps projection matmuls.
"""
import sys

sys.path.insert(0, "/opt/trn_rl_repo")

import ml_dtypes
import numpy as np

BF16 = ml_dtypes.bfloat16
GROUP = 128
N_HEADS = 16
EPS = 1e-8
B, T, C = 2, 2048, 1024
HD = 64
N_CORES = 8
HPC = N_HEADS // N_CORES  # 2 heads per core


# ---------------------------------------------------------------- host prep
def _ternary_quantize(w):
    O, I = w.shape
    g = w.reshape(O, I // GROUP, GROUP).astype(np.float32)
    scale = np.maximum(np.mean(np.abs(g), axis=-1, keepdims=True), EPS).astype(
        np.float32
    )
    wn = g / scale
    q = np.where(wn > 0.5, 1.0, np.where(wn < -0.5, -1.0, 0.0)).astype(np.float32)
    return (q * scale).reshape(O, I).astype(np.float32)


def _make_core_inputs(x, wq, wk, wv, wo, rope_cos, rope_sin):
    """Returns list of 8 per-core input dicts (bf16 device layouts)."""
    x = np.ascontiguousarray(x.astype(np.float32).reshape(B * T, C))
    wq_q = _ternary_quantize(wq)
    wk_q = _ternary_quantize(wk)
    wv_q = _ternary_quantize(wv)
    wo_q = _ternary_quantize(wo)

    xT = x.T  # [1024 c, 4096 t]
    xt_slab = np.ascontiguousarray(
        xT.reshape(8, 128, 8, 512).transpose(2, 1, 0, 3)
    ).astype(BF16)  # [s, p, kk, u]

    cosT = rope_cos.astype(np.float32).T  # [32, 2048]
    sinT = rope_sin.astype(np.float32).T
    cos_t = np.tile(cosT, (4, 1)).astype(BF16)
    sin_t = np.concatenate([-sinT, sinT, -sinT, sinT], axis=0).astype(BF16)
    # strict upper triangle (invalid: key k > query q within a diagonal block)
    tri = (np.arange(128)[None, :] < np.arange(128)[:, None]).astype(BF16)
    ident = np.eye(128, dtype=np.float32).astype(BF16)
    negid = (-1000.0 * np.eye(128, dtype=np.float32)).astype(BF16)

    maps = []
    for core in range(N_CORES):
        r0 = core * HPC * HD
        rows = slice(r0, r0 + HPC * HD)

        def w_lhsT(w_qq):
            wsT = w_qq[rows, :].T  # [1024 in, 128 d]
            return np.ascontiguousarray(
                wsT.reshape(8, 128, 128).transpose(1, 0, 2)
            ).astype(BF16)  # [p, kk, d]

        woc = wo_q[:, rows]  # [1024 o, 128 d]
        maps.append(
            {
                "xt": xt_slab,
                "wqT": w_lhsT(wq_q),
                "wkT": w_lhsT(wk_q),
                "wvT": w_lhsT(wv_q),
                "woC": np.ascontiguousarray(woc.T).astype(BF16),  # [128 d, 1024 o]
                "cos_t": cos_t,
                "sin_t": sin_t,
                "tri": tri,
                "ident": ident,
                "negid": negid,
            }
        )
    return maps


# ---------------------------------------------------------------- BIR post-pass
def _split_excess_waits(nc, max_waits=1):
    """walrus CoreV3 codegen rejects instructions with >1 sem wait; split the
    excess into preceding NoOps on the same engine."""
    import concourse.mybir as mybir

    for f in nc.m.functions:
        for bb in f.blocks:
            insts = bb.instructions
            i = 0
            while i < len(insts):
                ins = insts[i]
                si = ins.sync_info
                if si is not None and si.on_wait and len(si.on_wait) > max_waits:
                    waits = list(si.on_wait)
                    si.on_wait = waits[:max_waits]
                    rest = waits[max_waits:]
                    new_ops = []
                    for j in range(0, len(rest), max_waits):
                        new_ops.append(
                            mybir.InstNoOp(
                                name=nc.get_next_instruction_name(),
                                sync_info=mybir.SyncInfo(
                                    on_wait=rest[j : j + max_waits], on_update=[]
                                ),
                                bass_nofuse=True,
                                engine=ins.engine,
                            )
                        )
                    insts[i:i] = new_ops
                    i += len(new_ops)
                i += 1


# ---------------------------------------------------------------- device kernel
def _emit(nc, tc, d):
    import concourse.mybir as mybir
    from concourse.bass import ds, ts

    f32 = mybir.dt.float32
    bf16 = mybir.dt.bfloat16
    AF = mybir.ActivationFunctionType
    OP = mybir.AluOpType

    with nc.allow_low_precision(
        reason="bf16 activations; fp32 accum in PSUM; 2e-2 rel tol"
    ), tc.tile_pool(name="const", bufs=1) as cp, tc.tile_pool(
        name="persist", bufs=1
    ) as pp, tc.tile_pool(name="xt", bufs=3) as xtp, tc.tile_pool(
        name="sw", bufs=2
    ) as swp, tc.tile_pool(name="tmp", bufs=2) as tmpp, tc.tile_pool(
        name="E", bufs=3
    ) as epool, tc.tile_pool(name="y2n", bufs=2) as y2np, tc.tile_pool(
        name="rc", bufs=4
    ) as rcp, tc.tile_pool(name="scP", bufs=2, space="PSUM") as scp, tc.tile_pool(
        name="ypP", bufs=2, space="PSUM"
    ) as ypp, tc.tile_pool(name="scrP", bufs=2, space="PSUM") as scr:
        # ---- constants (xt slab 0 is DMA'd first, in the schedule below;
        # weight order matches first use so the PE start isn't DMA-gated)
        wq_t = cp.tile([128, 8, 128], bf16)
        wk_t = cp.tile([128, 8, 128], bf16)
        wv_t = cp.tile([128, 8, 128], bf16)
        woC = cp.tile([128, 1024], bf16)
        cos_sb = cp.tile([128, 2048], bf16)
        sin_sb = cp.tile([128, 2048], bf16)
        tri_t = cp.tile([128, 128], bf16)
        id_t = cp.tile([128, 128], bf16)
        nid_t = cp.tile([128, 128], bf16)

        def emit_consts_early():
            nc.sync.dma_start(wq_t[:], d["wqT"])
            nc.sync.dma_start(wk_t[:], d["wkT"])
            nc.sync.dma_start(cos_sb[:], d["cos_t"])
            nc.sync.dma_start(sin_sb[:], d["sin_t"])

        def emit_consts_late():
            nc.sync.dma_start(wv_t[:], d["wvT"])
            nc.sync.dma_start(tri_t[:], d["tri"])
            nc.sync.dma_start(id_t[:], d["ident"])
            nc.sync.dma_start(nid_t[:], d["negid"])
            nc.sync.dma_start(woC[:], d["woC"])

        # ---- persistent tensors
        qT = pp.tile([128, 4096], bf16)
        kT = pp.tile([128, 4096], bf16)
        vT = pp.tile([128, 4096], bf16)
        v_aug = pp.tile([128, 32 * 129], bf16)  # per key-block: [v0(64)|1|v1(64)]
        y2T = pp.tile([128, 4096], bf16)
        bo = [pp.tile([128, 16384], bf16, name=f"bo{b}") for b in range(2)]
        # bake the shared ones columns (col 64 of every 129-block)
        nc.gpsimd.memset(v_aug[:], 1.0)

        # ---- phase A granules: xt prefetch, then per-projection granules.
        # q/k granules include RoPE on the slab's columns; the v granule
        # includes the v transposes — so attention on a slab's queries can
        # start right after its three projection granules.
        _xt_tiles = {}

        def g_xt(s):
            def f():
                t = xtp.tile([128, 8, 512], bf16, tag="xt")
                nc.sync.dma_start(t[:], d["xt"][s])
                _xt_tiles[s] = t
            return f

        def _rope(tns, s):
            u = (s % 4) * 512  # within-batch token offset
            ccols = ds(u, 512)
            scols = ds(s * 512, 512)
            sw = swp.tile([128, 512], bf16, tag="sw")
            nc.sync.dma_start(sw[0:32, :], tns[32:64, scols])
            nc.sync.dma_start(sw[32:64, :], tns[0:32, scols])
            nc.sync.dma_start(sw[64:96, :], tns[96:128, scols])
            nc.sync.dma_start(sw[96:128, :], tns[64:96, scols])
            tmp = tmpp.tile([128, 512], bf16, tag="tmp")
            nc.vector.tensor_tensor(tmp[:], tns[:, scols], cos_sb[:, ccols], OP.mult)
            nc.vector.tensor_tensor(sw[:], sw[:], sin_sb[:, ccols], OP.mult)
            eng = nc.vector if s == 0 else nc.gpsimd
            eng.tensor_tensor(tns[:, scols], tmp[:], sw[:], OP.add)

        def g_proj(s, which, qk_on_act=False):
            def f():
                xt_t = _xt_tiles[s]
                w_t, dest = ((wq_t, qT), (wk_t, kT), (wv_t, vT))[which]
                ps = scr.tile([128, 512], f32, tag="scr")
                for kk in range(8):
                    nc.tensor.matmul(
                        ps[:],
                        w_t[:, kk, :],
                        xt_t[:, kk, :],
                        start=(kk == 0),
                        stop=(kk == 7),
                    )
                if qk_on_act and which < 2:
                    nc.scalar.copy(dest[:, ts(s, 512)], ps[:])
                else:
                    nc.vector.tensor_copy(dest[:, ts(s, 512)], ps[:])
                if which < 2:
                    _rope(dest, s)
                else:
                    for blk in range(4):
                        g = s * 4 + blk
                        tp = scr.tile([128, 128], bf16, tag="scr")
                        nc.tensor.transpose(
                            tp[:], vT[:, ds(g * 128, 128)], id_t[:]
                        )
                        nc.vector.tensor_copy(
                            v_aug[:, ds(g * 129, 64)], tp[:, 0:64]
                        )
                        nc.vector.tensor_copy(
                            v_aug[:, ds(g * 129 + 65, 64)], tp[:, 64:128]
                        )
            return f

        # ---- filler queue: independent PE work popped between attention
        # pipeline units so the in-order PE stream never starves while ACT
        # works through the exp backlog. Items are (slab_done_marker, fn).
        filler = []

        def pop_filler(n=1):
            for _ in range(n):
                if filler:
                    filler.pop(0)[1]()

        def drain_slab(s):
            while any(m is not None and m <= s for m, _ in filler):
                filler.pop(0)[1]()

        # ---- phase B: one (batch, 512-query-chunk) of attention
        def emit_chunk(b, qi, pop_every=2):
            nj = 4 * qi + 4
            npairs = nj // 2
            yp = [ypp.tile([128, 260], f32, tag="yp", name=f"yp{b}_{qi}_{h}")
                  for h in range(2)]
            q0 = b * 2048 + qi * 512

            units = [(p, h) for p in range(npairs) for h in range(2)]

            def emit_sc(p, h):
                sc = scp.tile([128, 1024], f32, tag="sc", name=f"sc{b}_{qi}_{p}_{h}")
                for jj in range(2):
                    j = 2 * p + jj
                    dlt0 = max(j * 128 - qi * 512, 0)
                    nc.tensor.matmul(
                        sc[:, ds(jj * 512 + dlt0, 512 - dlt0)],
                        kT[64 * h : 64 * h + 64, ds(b * 2048 + j * 128, 128)],
                        qT[64 * h : 64 * h + 64, ds(q0 + dlt0, 512 - dlt0)],
                        start=True,
                        stop=True,
                    )
                    if j >= 4 * qi:
                        # causal mask: add -1000 to the strict upper triangle
                        # of the diagonal block so exp() flushes it to zero
                        qbl = j - 4 * qi
                        nc.tensor.matmul(
                            sc[:, ds(jj * 512 + qbl * 128, 128)],
                            nid_t[:],
                            tri_t[:],
                            start=False,
                            stop=True,
                            skip_group_check=True,
                        )
                return sc

            def emit_rest(p, h, sc):
                # exp (trim the diagonal pairs so unwritten PSUM is never read)
                E = epool.tile([128, 1024], bf16, tag="E")
                j0, j1 = 2 * p, 2 * p + 1
                d0 = max(j0 * 128 - qi * 512, 0)
                d1 = max(j1 * 128 - qi * 512, 0)
                if d0 == 0 and d1 == 0:
                    nc.scalar.activation(E[:], sc[:], AF.Exp, scale=0.125)
                else:
                    nc.scalar.activation(
                        E[:, ds(d0, 512 - d0)], sc[:, ds(d0, 512 - d0)],
                        AF.Exp, scale=0.125,
                    )
                    nc.scalar.activation(
                        E[:, ds(512 + d1, 512 - d1)], sc[:, ds(512 + d1, 512 - d1)],
                        AF.Exp, scale=0.125,
                    )
                # pv: E block is the stationary operand -> out [q, d_aug]
                for jj, j in ((0, j0), (1, j1)):
                    g = b * 16 + j
                    for qbl in range(4):
                        qb_g = qi * 4 + qbl
                        if qb_g < j:
                            continue
                        nc.tensor.matmul(
                            yp[h][:, ds(qbl * 65, 65)],
                            E[:, ds(jj * 512 + qbl * 128, 128)],
                            v_aug[:, ds(g * 129 + 64 * h, 65)],
                            start=(j == 0),
                            stop=(j == qb_g),
                            skip_group_check=True,
                        )

            prev = None
            for ui, u in enumerate(units):
                sc = emit_sc(*u)
                if prev is not None:
                    emit_rest(prev[0], prev[1], prev[2])
                prev = (u[0], u[1], sc)
                if ui % pop_every == pop_every - 1:
                    pop_filler(1)
            emit_rest(prev[0], prev[1], prev[2])

            # epilogue: normalize by the denominator column, then transpose
            y2n = y2np.tile([128, 512], bf16, tag="y2n")
            for h in range(2):
                ypr = yp[h][:].rearrange("p (a c) -> p a c", a=4)
                rc = rcp.tile([128, 4], f32, tag="rc")
                den = ypr[:, :, 64:65] if h == 0 else ypr[:, :, 0:1]
                nc.vector.reciprocal(rc[:], den)
                data = ypr[:, :, 0:64] if h == 0 else ypr[:, :, 1:65]
                rcb = rc[:].unsqueeze(2).broadcast_to([128, 4, 64])
                outap = y2n[:].rearrange("p (a c) -> p a c", a=4)[
                    :, :, 64 * h : 64 * h + 64
                ]
                nc.vector.tensor_tensor(outap, data, rcb, OP.mult)
            for qbl in range(4):
                tp = scr.tile([128, 128], bf16, tag="scr")
                nc.tensor.transpose(tp[:], y2n[:, ds(qbl * 128, 128)], id_t[:])
                nc.vector.tensor_copy(
                    y2T[:, ds(b * 2048 + (qi * 4 + qbl) * 128, 128)], tp[:]
                )

        # ---- phase C: output projection granules (one token-block each)
        _copy_ctr = [0]

        def g_outblk(b, tb, split=False):
            def f():
                for oc in range(2):
                    op = scr.tile([128, 512], f32, tag="scr")
                    nc.tensor.matmul(
                        op[:],
                        y2T[:, ds(b * 2048 + tb * 128, 128)],
                        woC[:, ds(oc * 512, 512)],
                        start=True,
                        stop=True,
                    )
                    o0 = tb * 1024 + oc * 512
                    if split:
                        nc.vector.tensor_copy(bo[b][:, ds(o0, 256)], op[:, 0:256])
                        nc.scalar.copy(bo[b][:, ds(o0 + 256, 256)], op[:, 256:512])
                    else:
                        nc.vector.tensor_copy(bo[b][:, ds(o0, 512)], op[:])
            return f

        def g_outdma(b, grp, ntb=4):
            def f():
                dram = d["outp"][
                    ds(b * 2048 + grp * ntb * 128, ntb * 128), :
                ].rearrange("(a p) c -> p a c", p=128)
                src = bo[b][:, ds(grp * ntb * 1024, ntb * 1024)].rearrange(
                    "p (a c) -> p a c", a=ntb
                )
                nc.sync.dma_start(dram, src)
            return f

        # ---------------- emission schedule ----------------
        # slab s feeds chunk (s//4, s%4); attention starts right after slab 0.
        g_xt(0)()
        emit_consts_early()
        g_xt(1)()
        emit_consts_late()
        for w in range(3):
            g_proj(0, w, qk_on_act=True)()
        # filler: remaining slabs (xt prefetched one slab ahead) ...
        for s in range(1, 8):
            if s + 1 < 8:
                filler.append((None, g_xt(s + 1)))
            for w in range(3):
                filler.append((s, g_proj(s, w, qk_on_act=(s == 1))))
        # ... then batch-0 out-projection blocks
        for grp in range(4):
            for tb in range(grp * 4, grp * 4 + 4):
                filler.append((None, g_outblk(0, tb)))
            filler.append((None, g_outdma(0, grp)))

        for qi in range(4):
            drain_slab(qi)
            emit_chunk(0, qi)
        for qi in range(4):
            drain_slab(4 + qi)
            emit_chunk(1, qi, pop_every=1)
            if qi < 3:
                for tb in range(qi * 4, qi * 4 + 4):
                    filler.append((None, g_outblk(1, tb)))
                    if tb % 2 == 1:
                        filler.append((None, g_outdma(1, tb // 2, ntb=2)))
        pop_filler(len(filler))
        for tb in range(12, 16):
            g_outblk(1, tb, split=True)()
            if tb % 2 == 1:
                g_outdma(1, tb // 2, ntb=2)()


_NC_CACHE = {}


def _build():
    if "nc" in _NC_CACHE:
        return _NC_CACHE["nc"]
    import concourse.bass as bass
    import concourse.mybir as mybir
    import concourse.tile as tile

    bf16 = mybir.dt.bfloat16
    nc = bass.Bass("TRN2", target_bir_lowering=False, debug=False, num_devices=1)
    d = {
        "xt": nc.dram_tensor("xt", [8, 128, 8, 512], bf16, kind="ExternalInput").ap(),
        "wqT": nc.dram_tensor("wqT", [128, 8, 128], bf16, kind="ExternalInput").ap(),
        "wkT": nc.dram_tensor("wkT", [128, 8, 128], bf16, kind="ExternalInput").ap(),
        "wvT": nc.dram_tensor("wvT", [128, 8, 128], bf16, kind="ExternalInput").ap(),
        "woC": nc.dram_tensor("woC", [128, 1024], bf16, kind="ExternalInput").ap(),
        "cos_t": nc.dram_tensor("cos_t", [128, 2048], bf16, kind="ExternalInput").ap(),
        "sin_t": nc.dram_tensor("sin_t", [128, 2048], bf16, kind="ExternalInput").ap(),
        "tri": nc.dram_tensor("tri", [128, 128], bf16, kind="ExternalInput").ap(),
        "ident": nc.dram_tensor("ident", [128, 128], bf16, kind="ExternalInput").ap(),
        "negid": nc.dram_tensor("negid", [128, 128], bf16, kind="ExternalInput").ap(),
        "outp": nc.dram_tensor("outp", [4096, 1024], bf16, kind="ExternalOutput").ap(),
    }
    with tile.TileContext(nc) as tc:
        _emit(nc, tc, d)
    _split_excess_waits(nc)
    _NC_CACHE["nc"] = nc
    return nc


def kernel(x, wq, wk, wv, wo, rope_cos, rope_sin):
    from concourse import bass_utils

    x, wq, wk, wv, wo, rope_cos, rope_sin = (
        np.asarray(a, dtype=np.float32)
        for a in (x, wq, wk, wv, wo, rope_cos, rope_sin)
    )
    in_maps = _make_core_inputs(x, wq, wk, wv, wo, rope_cos, rope_sin)
    nc = _build()
    res = bass_utils.run_bass_kernel_spmd(nc, in_maps, core_ids=list(range(N_CORES)))
    total = np.zeros((B * T, C), np.float32)
    for i in range(N_CORES):
        total += res.results[i]["outp"].astype(np.float32)
    return total.reshape(B, T, C).astype(np.float32)


# revision 16
# speedup vs baseline: 1.1633x; 1.0313x over previous
"""Trainium2 Bass kernel for nn_CausalSelfAttention (BitLinear QKV/O + RoPE + causal attn).

Sharding: 2 heads x 2 batches per core (head-parallel), bf16 throughout.
Per core: q/k/v projections ([d, t] layout, bf16 matmuls, fp32 PSUM), RoPE via
DMA partition shuffle + DVE/Pool elementwise, scores in [k, q] layout, exp on
ACT (scale=0.125) -> E bf16, pv as many small-N matmuls with E as the
stationary operand giving [q, d_aug] output whose 65th column is the softmax
denominator (ones column baked into v_aug), per-partition normalization via
DVE tensor_tensor with a broadcast reciprocal, PE transposes back to [d, t],
column-sharded output projection producing a bf16 partial [4096, 1024] that
the host sums across cores.

Engine budget per core (TimelineSim model): PE ~102us (at the matmul floor:
cost = out-free-size x 0.417ns/row), ACT ~85us (exp + some copies), DVE ~85us
(copies, rope, epilogue), Pool ~45us (tri masks, rope adds). Emission order
interleaves batch-0 attention with batch-1 projections (and batch-1 attention
with batch-0 out-projection) so exp overlaps projection matmuls.
"""
import sys

sys.path.insert(0, "/opt/trn_rl_repo")

import ml_dtypes
import numpy as np

BF16 = ml_dtypes.bfloat16
GROUP = 128
N_HEADS = 16
EPS = 1e-8
B, T, C = 2, 2048, 1024
HD = 64
N_CORES = 8
HPC = N_HEADS // N_CORES  # 2 heads per core


# ---------------------------------------------------------------- host prep
def _ternary_quantize(w):
    O, I = w.shape
    g = w.reshape(O, I // GROUP, GROUP).astype(np.float32)
    scale = np.maximum(np.mean(np.abs(g), axis=-1, keepdims=True), EPS).astype(
        np.float32
    )
    wn = g / scale
    q = np.where(wn > 0.5, 1.0, np.where(wn < -0.5, -1.0, 0.0)).astype(np.float32)
    return (q * scale).reshape(O, I).astype(np.float32)


def _make_core_inputs(x, wq, wk, wv, wo, rope_cos, rope_sin):
    """Returns list of 8 per-core input dicts (bf16 device layouts)."""
    x = np.ascontiguousarray(x.astype(np.float32).reshape(B * T, C))
    wq_q = _ternary_quantize(wq)
    wk_q = _ternary_quantize(wk)
    wv_q = _ternary_quantize(wv)
    wo_q = _ternary_quantize(wo)

    xT = x.T  # [1024 c, 4096 t]
    xt_slab = np.ascontiguousarray(
        xT.reshape(8, 128, 8, 512).transpose(2, 1, 0, 3)
    ).astype(BF16)  # [s, p, kk, u]

    cosT = rope_cos.astype(np.float32).T  # [32, 2048]
    sinT = rope_sin.astype(np.float32).T
    cos_t = np.tile(cosT, (4, 1)).astype(BF16)
    sin_t = np.concatenate([-sinT, sinT, -sinT, sinT], axis=0).astype(BF16)
    # strict upper triangle (invalid: key k > query q within a diagonal block)
    tri = (np.arange(128)[None, :] < np.arange(128)[:, None]).astype(BF16)
    ident = np.eye(128, dtype=np.float32).astype(BF16)
    negid = (-1000.0 * np.eye(128, dtype=np.float32)).astype(BF16)

    maps = []
    for core in range(N_CORES):
        r0 = core * HPC * HD
        rows = slice(r0, r0 + HPC * HD)

        def w_lhsT(w_qq):
            wsT = w_qq[rows, :].T  # [1024 in, 128 d]
            return np.ascontiguousarray(
                wsT.reshape(8, 128, 128).transpose(1, 0, 2)
            ).astype(BF16)  # [p, kk, d]

        woc = wo_q[:, rows]  # [1024 o, 128 d]
        maps.append(
            {
                "xt": xt_slab,
                "wqT": w_lhsT(wq_q),
                "wkT": w_lhsT(wk_q),
                "wvT": w_lhsT(wv_q),
                "woC": np.ascontiguousarray(woc.T).astype(BF16),  # [128 d, 1024 o]
                "cos_t": cos_t,
                "sin_t": sin_t,
                "tri": tri,
                "ident": ident,
                "negid": negid,
            }
        )
    return maps


# ---------------------------------------------------------------- BIR post-pass
def _split_excess_waits(nc, max_waits=1):
    """walrus CoreV3 codegen rejects instructions with >1 sem wait; split the
    excess into preceding NoOps on the same engine."""
    import concourse.mybir as mybir

    for f in nc.m.functions:
        for bb in f.blocks:
            insts = bb.instructions
            i = 0
            while i < len(insts):
                ins = insts[i]
                si = ins.sync_info
                if si is not None and si.on_wait and len(si.on_wait) > max_waits:
                    waits = list(si.on_wait)
                    si.on_wait = waits[:max_waits]
                    rest = waits[max_waits:]
                    new_ops = []
                    for j in range(0, len(rest), max_waits):
                        new_ops.append(
                            mybir.InstNoOp(
                                name=nc.get_next_instruction_name(),
                                sync_info=mybir.SyncInfo(
                                    on_wait=rest[j : j + max_waits], on_update=[]
                                ),
                                bass_nofuse=True,
                                engine=ins.engine,
                            )
                        )
                    insts[i:i] = new_ops
                    i += len(new_ops)
                i += 1


# ---------------------------------------------------------------- device kernel
def _emit(nc, tc, d):
    import concourse.mybir as mybir
    from concourse.bass import ds, ts

    f32 = mybir.dt.float32
    bf16 = mybir.dt.bfloat16
    AF = mybir.ActivationFunctionType
    OP = mybir.AluOpType

    with nc.allow_low_precision(
        reason="bf16 activations; fp32 accum in PSUM; 2e-2 rel tol"
    ), tc.tile_pool(name="const", bufs=1) as cp, tc.tile_pool(
        name="persist", bufs=1
    ) as pp, tc.tile_pool(name="xt", bufs=3) as xtp, tc.tile_pool(
        name="sw", bufs=2
    ) as swp, tc.tile_pool(name="tmp", bufs=2) as tmpp, tc.tile_pool(
        name="E", bufs=3
    ) as epool, tc.tile_pool(name="y2n", bufs=2) as y2np, tc.tile_pool(
        name="rc", bufs=4
    ) as rcp, tc.tile_pool(name="scP", bufs=2, space="PSUM") as scp, tc.tile_pool(
        name="ypP", bufs=2, space="PSUM"
    ) as ypp, tc.tile_pool(name="scrP", bufs=2, space="PSUM") as scr:
        # ---- constants (xt slab 0 is DMA'd first, in the schedule below;
        # weight order matches first use so the PE start isn't DMA-gated)
        wq_t = cp.tile([128, 8, 128], bf16)
        wk_t = cp.tile([128, 8, 128], bf16)
        wv_t = cp.tile([128, 8, 128], bf16)
        woC = cp.tile([128, 1024], bf16)
        cos_sb = cp.tile([128, 2048], bf16)
        sin_sb = cp.tile([128, 2048], bf16)
        tri_t = cp.tile([128, 128], bf16)
        id_t = cp.tile([128, 128], bf16)
        nid_t = cp.tile([128, 128], bf16)

        def emit_consts_early():
            nc.sync.dma_start(wq_t[:], d["wqT"])
            nc.sync.dma_start(wk_t[:], d["wkT"])
            nc.sync.dma_start(cos_sb[:], d["cos_t"])
            nc.sync.dma_start(sin_sb[:], d["sin_t"])

        def emit_consts_late():
            nc.sync.dma_start(wv_t[:], d["wvT"])
            nc.sync.dma_start(tri_t[:], d["tri"])
            nc.sync.dma_start(id_t[:], d["ident"])
            nc.sync.dma_start(nid_t[:], d["negid"])
            nc.sync.dma_start(woC[:], d["woC"])

        # ---- persistent tensors
        qT = pp.tile([128, 4096], bf16)
        kT = pp.tile([128, 4096], bf16)
        vT = pp.tile([128, 4096], bf16)
        v_aug = pp.tile([128, 32 * 129], bf16)  # per key-block: [v0(64)|1|v1(64)]
        y2T = pp.tile([128, 4096], bf16)
        bo = [pp.tile([128, 16384], bf16, name=f"bo{b}") for b in range(2)]
        # bake the shared ones columns (col 64 of every 129-block)
        nc.gpsimd.memset(v_aug[:], 1.0)

        # ---- phase A granules: xt prefetch, then per-projection granules.
        # q/k granules include RoPE on the slab's columns; the v granule
        # includes the v transposes — so attention on a slab's queries can
        # start right after its three projection granules.
        _xt_tiles = {}

        def g_xt(s):
            def f():
                t = xtp.tile([128, 8, 512], bf16, tag="xt")
                nc.sync.dma_start(t[:], d["xt"][s])
                _xt_tiles[s] = t
            return f

        def _rope(tns, s):
            u = (s % 4) * 512  # within-batch token offset
            ccols = ds(u, 512)
            scols = ds(s * 512, 512)
            sw = swp.tile([128, 512], bf16, tag="sw")
            nc.sync.dma_start(sw[0:32, :], tns[32:64, scols])
            nc.sync.dma_start(sw[32:64, :], tns[0:32, scols])
            nc.sync.dma_start(sw[64:96, :], tns[96:128, scols])
            nc.sync.dma_start(sw[96:128, :], tns[64:96, scols])
            tmp = tmpp.tile([128, 512], bf16, tag="tmp")
            nc.vector.tensor_tensor(tmp[:], tns[:, scols], cos_sb[:, ccols], OP.mult)
            nc.vector.tensor_tensor(sw[:], sw[:], sin_sb[:, ccols], OP.mult)
            eng = nc.vector if s == 0 else nc.gpsimd
            eng.tensor_tensor(tns[:, scols], tmp[:], sw[:], OP.add)

        def g_proj(s, which, qk_on_act=False):
            def f():
                xt_t = _xt_tiles[s]
                w_t, dest = ((wq_t, qT), (wk_t, kT), (wv_t, vT))[which]
                ps = scr.tile([128, 512], f32, tag="scr")
                for kk in range(8):
                    nc.tensor.matmul(
                        ps[:],
                        w_t[:, kk, :],
                        xt_t[:, kk, :],
                        start=(kk == 0),
                        stop=(kk == 7),
                    )
                if qk_on_act and which < 2:
                    nc.scalar.copy(dest[:, ts(s, 512)], ps[:])
                else:
                    nc.vector.tensor_copy(dest[:, ts(s, 512)], ps[:])
                if which < 2:
                    _rope(dest, s)
                else:
                    for blk in range(4):
                        g = s * 4 + blk
                        tp = scr.tile([128, 128], bf16, tag="scr")
                        nc.tensor.transpose(
                            tp[:], vT[:, ds(g * 128, 128)], id_t[:]
                        )
                        nc.vector.tensor_copy(
                            v_aug[:, ds(g * 129, 64)], tp[:, 0:64]
                        )
                        nc.vector.tensor_copy(
                            v_aug[:, ds(g * 129 + 65, 64)], tp[:, 64:128]
                        )
            return f

        # ---- filler queue: independent PE work popped between attention
        # pipeline units so the in-order PE stream never starves while ACT
        # works through the exp backlog. Items are (slab_done_marker, fn).
        filler = []

        def pop_filler(n=1):
            for _ in range(n):
                if filler:
                    filler.pop(0)[1]()

        def drain_slab(s):
            while any(m is not None and m <= s for m, _ in filler):
                filler.pop(0)[1]()

        # ---- phase B: one (batch, 512-query-chunk) of attention
        def emit_chunk(b, qi, pop_every=2):
            nj = 4 * qi + 4
            npairs = nj // 2
            yp = [ypp.tile([128, 260], f32, tag="yp", name=f"yp{b}_{qi}_{h}")
                  for h in range(2)]
            q0 = b * 2048 + qi * 512

            units = [(p, h) for p in range(npairs) for h in range(2)]

            def emit_sc(p, h):
                sc = scp.tile([128, 1024], f32, tag="sc", name=f"sc{b}_{qi}_{p}_{h}")
                for jj in range(2):
                    j = 2 * p + jj
                    dlt0 = max(j * 128 - qi * 512, 0)
                    nc.tensor.matmul(
                        sc[:, ds(jj * 512 + dlt0, 512 - dlt0)],
                        kT[64 * h : 64 * h + 64, ds(b * 2048 + j * 128, 128)],
                        qT[64 * h : 64 * h + 64, ds(q0 + dlt0, 512 - dlt0)],
                        start=True,
                        stop=True,
                    )
                    if j >= 4 * qi:
                        # causal mask: add -1000 to the strict upper triangle
                        # of the diagonal block so exp() flushes it to zero
                        qbl = j - 4 * qi
                        nc.tensor.matmul(
                            sc[:, ds(jj * 512 + qbl * 128, 128)],
                            nid_t[:],
                            tri_t[:],
                            start=False,
                            stop=True,
                            skip_group_check=True,
                        )
                return sc

            def emit_rest(p, h, sc):
                # exp (trim the diagonal pairs so unwritten PSUM is never read)
                E = epool.tile([128, 1024], bf16, tag="E")
                j0, j1 = 2 * p, 2 * p + 1
                d0 = max(j0 * 128 - qi * 512, 0)
                d1 = max(j1 * 128 - qi * 512, 0)
                if d0 == 0 and d1 == 0:
                    nc.scalar.activation(E[:], sc[:], AF.Exp, scale=0.125)
                else:
                    nc.scalar.activation(
                        E[:, ds(d0, 512 - d0)], sc[:, ds(d0, 512 - d0)],
                        AF.Exp, scale=0.125,
                    )
                    nc.scalar.activation(
                        E[:, ds(512 + d1, 512 - d1)], sc[:, ds(512 + d1, 512 - d1)],
                        AF.Exp, scale=0.125,
                    )
                # pv: E block is the stationary operand -> out [q, d_aug]
                for jj, j in ((0, j0), (1, j1)):
                    g = b * 16 + j
                    for qbl in range(4):
                        qb_g = qi * 4 + qbl
                        if qb_g < j:
                            continue
                        nc.tensor.matmul(
                            yp[h][:, ds(qbl * 65, 65)],
                            E[:, ds(jj * 512 + qbl * 128, 128)],
                            v_aug[:, ds(g * 129 + 64 * h, 65)],
                            start=(j == 0),
                            stop=(j == qb_g),
                            skip_group_check=True,
                        )

            prev = None
            for ui, u in enumerate(units):
                sc = emit_sc(*u)
                if prev is not None:
                    emit_rest(prev[0], prev[1], prev[2])
                prev = (u[0], u[1], sc)
                if ui % pop_every == pop_every - 1:
                    pop_filler(1)
            emit_rest(prev[0], prev[1], prev[2])

            # epilogue: normalize by the denominator column, then transpose
            y2n = y2np.tile([128, 512], bf16, tag="y2n")
            for h in range(2):
                ypr = yp[h][:].rearrange("p (a c) -> p a c", a=4)
                rc = rcp.tile([128, 4], f32, tag="rc")
                den = ypr[:, :, 64:65] if h == 0 else ypr[:, :, 0:1]
                nc.vector.reciprocal(rc[:], den)
                data = ypr[:, :, 0:64] if h == 0 else ypr[:, :, 1:65]
                rcb = rc[:].unsqueeze(2).broadcast_to([128, 4, 64])
                outap = y2n[:].rearrange("p (a c) -> p a c", a=4)[
                    :, :, 64 * h : 64 * h + 64
                ]
                nc.vector.tensor_tensor(outap, data, rcb, OP.mult)
            for qbl in range(4):
                tp = scr.tile([128, 128], bf16, tag="scr")
                nc.tensor.transpose(tp[:], y2n[:, ds(qbl * 128, 128)], id_t[:])
                nc.vector.tensor_copy(
                    y2T[:, ds(b * 2048 + (qi * 4 + qbl) * 128, 128)], tp[:]
                )

        # ---- phase C: output projection granules (one token-block each)
        _copy_ctr = [0]

        def g_outblk(b, tb, split=False):
            def f():
                for oc in range(2):
                    op = scr.tile([128, 512], f32, tag="scr")
                    nc.tensor.matmul(
                        op[:],
                        y2T[:, ds(b * 2048 + tb * 128, 128)],
                        woC[:, ds(oc * 512, 512)],
                        start=True,
                        stop=True,
                    )
                    o0 = tb * 1024 + oc * 512
                    if split:
                        nc.vector.tensor_copy(bo[b][:, ds(o0, 256)], op[:, 0:256])
                        nc.scalar.copy(bo[b][:, ds(o0 + 256, 256)], op[:, 256:512])
                    else:
                        nc.vector.tensor_copy(bo[b][:, ds(o0, 512)], op[:])
            return f

        def g_outdma(b, grp, ntb=4):
            def f():
                dram = d["outp"][
                    ds(b * 2048 + grp * ntb * 128, ntb * 128), :
                ].rearrange("(a p) c -> p a c", p=128)
                src = bo[b][:, ds(grp * ntb * 1024, ntb * 1024)].rearrange(
                    "p (a c) -> p a c", a=ntb
                )
                nc.sync.dma_start(dram, src)
            return f

        # ---------------- emission schedule ----------------
        # slab s feeds chunk (s//4, s%4); attention starts right after slab 0.
        g_xt(0)()
        emit_consts_early()
        g_xt(1)()
        emit_consts_late()
        for w in range(3):
            g_proj(0, w, qk_on_act=True)()
        g_xt(2)()
        for w in range(3):
            g_proj(1, w, qk_on_act=True)()
        g_xt(3)()
        for w in range(3):
            g_proj(2, w)()
        # filler: remaining slabs (xt prefetched one slab ahead)
        for s in range(3, 8):
            if s + 1 < 8:
                filler.append((None, g_xt(s + 1)))
            for w in range(3):
                filler.append((s, g_proj(s, w)))

        for qi in range(4):
            drain_slab(qi)
            emit_chunk(0, qi, pop_every=1)
            # this chunk's out-projection is ready now; front-insert it so it
            # lands in the DVE-idle early region instead of piling up late
            og = [(None, g_outblk(0, qi * 4 + tbl)) for tbl in range(4)]
            og.append((None, g_outdma(0, qi)))
            filler[0:0] = og
        for qi in range(4):
            drain_slab(4 + qi)
            emit_chunk(1, qi, pop_every=1)
            if qi < 3:
                og = []
                for tb in range(qi * 4, qi * 4 + 4):
                    og.append((None, g_outblk(1, tb)))
                    if tb % 2 == 1:
                        og.append((None, g_outdma(1, tb // 2, ntb=2)))
                filler[0:0] = og
        pop_filler(len(filler))
        for tb in range(12, 16):
            g_outblk(1, tb, split=True)()
            if tb % 2 == 1:
                g_outdma(1, tb // 2, ntb=2)()


_NC_CACHE = {}


def _build():
    if "nc" in _NC_CACHE:
        return _NC_CACHE["nc"]
    import concourse.bass as bass
    import concourse.mybir as mybir
    import concourse.tile as tile

    bf16 = mybir.dt.bfloat16
    nc = bass.Bass("TRN2", target_bir_lowering=False, debug=False, num_devices=1)
    d = {
        "xt": nc.dram_tensor("xt", [8, 128, 8, 512], bf16, kind="ExternalInput").ap(),
        "wqT": nc.dram_tensor("wqT", [128, 8, 128], bf16, kind="ExternalInput").ap(),
        "wkT": nc.dram_tensor("wkT", [128, 8, 128], bf16, kind="ExternalInput").ap(),
        "wvT": nc.dram_tensor("wvT", [128, 8, 128], bf16, kind="ExternalInput").ap(),
        "woC": nc.dram_tensor("woC", [128, 1024], bf16, kind="ExternalInput").ap(),
        "cos_t": nc.dram_tensor("cos_t", [128, 2048], bf16, kind="ExternalInput").ap(),
        "sin_t": nc.dram_tensor("sin_t", [128, 2048], bf16, kind="ExternalInput").ap(),
        "tri": nc.dram_tensor("tri", [128, 128], bf16, kind="ExternalInput").ap(),
        "ident": nc.dram_tensor("ident", [128, 128], bf16, kind="ExternalInput").ap(),
        "negid": nc.dram_tensor("negid", [128, 128], bf16, kind="ExternalInput").ap(),
        "outp": nc.dram_tensor("outp", [4096, 1024], bf16, kind="ExternalOutput").ap(),
    }
    with tile.TileContext(nc) as tc:
        _emit(nc, tc, d)
    _split_excess_waits(nc)
    _NC_CACHE["nc"] = nc
    return nc


def kernel(x, wq, wk, wv, wo, rope_cos, rope_sin):
    from concourse import bass_utils

    x, wq, wk, wv, wo, rope_cos, rope_sin = (
        np.asarray(a, dtype=np.float32)
        for a in (x, wq, wk, wv, wo, rope_cos, rope_sin)
    )
    in_maps = _make_core_inputs(x, wq, wk, wv, wo, rope_cos, rope_sin)
    nc = _build()
    res = bass_utils.run_bass_kernel_spmd(nc, in_maps, core_ids=list(range(N_CORES)))
    total = np.zeros((B * T, C), np.float32)
    for i in range(N_CORES):
        total += res.results[i]["outp"].astype(np.float32)
    return total.reshape(B, T, C).astype(np.float32)
